# revision 23
# baseline (speedup 1.0000x reference)
"""TRN2 Bass kernel for BEiT-style attention (nn_Attention_27771258536423).

Strategy: data-parallel over batch across 8 NeuronCores (8 batches/core).
Per core (all matmuls bf16, psum f32):
  - rel-pos bias precomputed on host, shipped TRANSPOSED and pair-packed.
  - qkv: q,k channel-major [ch, tok] bf16 (q pre-scaled+biased via folded
    weights); v token-major [tok, 65*12] bf16 with a ones column per head
    (denominator rides along in the AV matmul).
  - attention per (batch, head-pair): S^T = k^T q directly (no transposes
    anywhere: exp(S^T + bias^T) IS E^T, the AV moving operand).
    AV: out[65, 197] = V_aug^T E^T -> rows 0:64 = attn_out^T (channel-major),
    row 64 = softmax denominator. reciprocal (DVE) -> partition_broadcast
    (gpsimd) -> multiply (DVE) into attn_outT bf16.
  - proj per batch, interleaved into the attention stream as PE filler
    together with next-chunk qkv matmuls (covers softmax latency).
"""
import sys

sys.path.insert(0, '/opt/trn_rl_repo')

import numpy as np
import ml_dtypes

import concourse.bass as bass
import concourse.mybir as mybir
import concourse.tile as tile
from concourse import bacc

dt = mybir.dt
BF16 = ml_dtypes.bfloat16

DIM = 768
NH = 12
HD = 64
N_TOK = 197
SCALE = HD ** (-0.5)
CHUNK = 2 * N_TOK          # 394 tokens = 2 batches per qkv chunk
N1C = [(0, 128), (128, 69)]  # token chunks within one batch (n2 chunks)

_cache = {}


def _ap(t, offset, ap):
    return bass.AP(tensor=t.tensor if hasattr(t, 'tensor') else t,
                   offset=offset, ap=ap)


def build_program(nb, debug=False, feats="scores,bias,exp,av,rt,proj,fill"):
    F = set(feats.split(",")) if feats else set()
    """nb = batches per core (8). Returns compiled Bacc."""
    assert nb % 2 == 0
    ntok = nb * N_TOK
    nchunks = nb // 2

    nc = bacc.Bacc(None)
    if debug:
        dbg_qk_d = nc.dram_tensor("dbg_qk", [12 * 128, ntok], dt.bfloat16,
                                  kind="ExternalOutput")
        dbg_v_d = nc.dram_tensor("dbg_v", [2 * 128, 65 * NH], dt.bfloat16,
                                 kind="ExternalOutput")
        dbg_ss0_d = nc.dram_tensor("dbg_ss0", [128, 394], dt.float32,
                                   kind="ExternalOutput")
        dbg_ss1_d = nc.dram_tensor("dbg_ss1", [69, 394], dt.float32,
                                   kind="ExternalOutput")
        dbg_et0_d = nc.dram_tensor("dbg_et0", [128, 394], dt.bfloat16,
                                   kind="ExternalOutput")
        dbg_et1_d = nc.dram_tensor("dbg_et1", [69, 394], dt.bfloat16,
                                   kind="ExternalOutput")
        dbg_rc_d = nc.dram_tensor("dbg_rc", [1, 394], dt.float32,
                                  kind="ExternalOutput")
        dbg_rb_d = nc.dram_tensor("dbg_rb", [64, 394], dt.float32,
                                  kind="ExternalOutput")
        dbg_ao_d = nc.dram_tensor("dbg_ao", [6 * 128, ntok], dt.bfloat16,
                                  kind="ExternalOutput")

    xTb_d = nc.dram_tensor("xTb", [DIM, ntok], dt.bfloat16, kind="ExternalInput")
    wqkT_d = nc.dram_tensor("wqkT", [DIM, 2 * DIM], dt.bfloat16, kind="ExternalInput")
    wvT_d = nc.dram_tensor("wvT", [DIM, DIM], dt.bfloat16, kind="ExternalInput")
    wpT_d = nc.dram_tensor("wpT", [DIM, DIM], dt.bfloat16, kind="ExternalInput")
    qb2_d = nc.dram_tensor("qb2", [128, 6], dt.float32, kind="ExternalInput")
    vb_d = nc.dram_tensor("vb", [DIM], dt.float32, kind="ExternalInput")
    pb_d = nc.dram_tensor("pb", [DIM], dt.float32, kind="ExternalInput")
    bT0_d = nc.dram_tensor("bT0", [6 * 128, 394], dt.bfloat16, kind="ExternalInput")
    bT1_d = nc.dram_tensor("bT1", [6 * 69, 394], dt.bfloat16, kind="ExternalInput")
    y_d = nc.dram_tensor("y", [ntok, DIM], dt.float32, kind="ExternalOutput")

    Exp = mybir.ActivationFunctionType.Exp

    with tile.TileContext(nc) as tc:
        import contextlib
        with contextlib.ExitStack() as stk:
            consts = stk.enter_context(tc.tile_pool(name="consts", bufs=1))
            wpool = stk.enter_context(tc.tile_pool(name="wpool", bufs=1))
            xp = stk.enter_context(tc.tile_pool(name="xp", bufs=1))
            qkp = stk.enter_context(tc.tile_pool(name="qkp", bufs=1))
            vp = stk.enter_context(tc.tile_pool(name="vp", bufs=1))
            aop = stk.enter_context(tc.tile_pool(name="aop", bufs=1))
            ss0p = stk.enter_context(tc.tile_pool(name="ss0p", bufs=3))
            ss1p = stk.enter_context(tc.tile_pool(name="ss1p", bufs=3))
            et0p = stk.enter_context(tc.tile_pool(name="et0p", bufs=3))
            et1p = stk.enter_context(tc.tile_pool(name="et1p", bufs=3))
            rcp = stk.enter_context(tc.tile_pool(name="rcp", bufs=4))
            rbp = stk.enter_context(tc.tile_pool(name="rbp", bufs=8))
            anump = stk.enter_context(tc.tile_pool(name="anump", bufs=8))
            rwp = stk.enter_context(tc.tile_pool(name="rwp", bufs=2))
            ysp = stk.enter_context(tc.tile_pool(name="ysp", bufs=3))
            mmps = stk.enter_context(tc.tile_pool(name="mmps", bufs=5, space="PSUM"))
            avps = stk.enter_context(tc.tile_pool(name="avps", bufs=3, space="PSUM"))
            dramp = stk.enter_context(tc.tile_pool(name="dramp", bufs=2, space="DRAM"))

            # ---------- constant / weight DMAs ----------
            # x chunk 0 + q-weight tiles first (first matmuls need them)
            xb = [xp.tile([128, ntok], dt.bfloat16, name=f"xb{k}", tag=f"xb{k}")
                  for k in range(6)]
            for k in range(6):
                nc.sync.dma_start(out=xb[k][:, 0:CHUNK],
                                  in_=xTb_d[128 * k:128 * (k + 1), 0:CHUNK])
            wqk = [wpool.tile([128, 2 * DIM], dt.bfloat16, name=f"wqk{k}",
                              tag=f"wqk{k}") for k in range(6)]
            for k in range(6):   # q columns first
                nc.sync.dma_start(out=wqk[k][:, 0:DIM],
                                  in_=wqkT_d[128 * k:128 * (k + 1), 0:DIM])
            qb2_sb = consts.tile([128, 6], dt.float32, name="qb2", tag="qb2")
            nc.sync.dma_start(out=qb2_sb[:, :], in_=qb2_d[:, :])
            for k in range(6):   # k columns
                nc.sync.dma_start(out=wqk[k][:, DIM:2 * DIM],
                                  in_=wqkT_d[128 * k:128 * (k + 1), DIM:2 * DIM])
            wv = [wpool.tile([128, DIM], dt.bfloat16, name=f"wv{k}", tag=f"wv{k}")
                  for k in range(6)]
            for k in range(6):
                nc.sync.dma_start(out=wv[k][:, :],
                                  in_=wvT_d[128 * k:128 * (k + 1), :])
            vb_rep = consts.tile([128, DIM], dt.float32, name="vbrep", tag="vbrep")
            nc.sync.dma_start(out=vb_rep[:, :],
                              in_=_ap(vb_d, 0, [[0, 128], [1, DIM]]))
            # bias tiles (needed at attention time)
            bT0_sb = []
            bT1_sb = []
            for hp in range(6):
                b0 = consts.tile([128, 394], dt.bfloat16, name=f"bT0_{hp}",
                                 tag=f"bT0_{hp}")
                nc.sync.dma_start(out=b0[:, :],
                                  in_=bT0_d[128 * hp:128 * (hp + 1), :])
                bT0_sb.append(b0)
                b1 = consts.tile([69, 394], dt.bfloat16, name=f"bT1_{hp}",
                                 tag=f"bT1_{hp}")
                nc.sync.dma_start(out=b1[:, :],
                                  in_=bT1_d[69 * hp:69 * (hp + 1), :])
                bT1_sb.append(b1)
            # remaining x chunks
            for c in range(1, nchunks):
                for k in range(6):
                    nc.sync.dma_start(
                        out=xb[k][:, CHUNK * c:CHUNK * (c + 1)],
                        in_=xTb_d[128 * k:128 * (k + 1), CHUNK * c:CHUNK * (c + 1)])
            wp = [wpool.tile([128, DIM], dt.bfloat16, name=f"wp{k}", tag=f"wp{k}")
                  for k in range(6)]
            for k in range(6):
                nc.sync.dma_start(out=wp[k][:, :],
                                  in_=wpT_d[128 * k:128 * (k + 1), :])
            pb_rep = consts.tile([128, DIM], dt.float32, name="pbrep", tag="pbrep")
            nc.sync.dma_start(out=pb_rep[:, :],
                              in_=_ap(pb_d, 0, [[0, 128], [1, DIM]]))

            # ---------- persistent sbuf tiles ----------
            qk_sb = [qkp.tile([128, ntok], dt.bfloat16, name=f"qk{m}", tag=f"qk{m}")
                     for m in range(12)]  # 0-5 q (scaled+biased), 6-11 k
            # v: token-major, 65 cols per head (64 v + shared ones col)
            v_sb = [[vp.tile([n2c, 65 * NH], dt.bfloat16, name=f"v{b}_{ci}",
                             tag=f"v{b}_{ci}")
                     for ci, (n2o, n2c) in enumerate(N1C)] for b in range(nb)]
            ao = [aop.tile([128, ntok], dt.bfloat16, name=f"ao{m}", tag=f"ao{m}")
                  for m in range(6)]  # attn_out^T, head pair per tile

            # ---------- work-unit emitters ----------
            def emit_qk_m(c, m):
                """q/k projection for block m, token chunk c."""
                no = CHUNK * c
                ps = mmps.tile([128, 512], dt.float32, name="mm", tag="mm")
                col = 128 * m if m < 6 else DIM + 128 * (m - 6)
                for k in range(6):
                    nc.tensor.matmul(ps[:, 0:CHUNK],
                                     wqk[k][:, col:col + 128],
                                     xb[k][:, no:no + CHUNK],
                                     start=(k == 0), stop=(k == 5))
                if m < 6:
                    nc.vector.tensor_scalar(
                        out=qk_sb[m][:, no:no + CHUNK], in0=ps[:, 0:CHUNK],
                        scalar1=qb2_sb[:, m:m + 1], scalar2=None,
                        op0=mybir.AluOpType.add)
                else:
                    nc.vector.tensor_copy(qk_sb[m][:, no:no + CHUNK],
                                          ps[:, 0:CHUNK])

            def emit_v(b, ci, half):
                """v projection for batch b, token chunk ci, 384-col half."""
                n2o, n2c = N1C[ci]
                vt = v_sb[b][ci]
                if half == 0:  # ones columns once per tile
                    nc.vector.memset(
                        _ap(vt, vt.offset + 64,
                            [[vt.ap[0][0], n2c], [65, NH]]), 1.0)
                ps = mmps.tile([128, 512], dt.float32, name="mm", tag="mm")
                for k in range(6):
                    nc.tensor.matmul(
                        ps[0:n2c, 0:384],
                        xb[k][:, N_TOK * b + n2o:N_TOK * b + n2o + n2c],
                        wv[k][:, 384 * half:384 * (half + 1)],
                        start=(k == 0), stop=(k == 5))
                nc.vector.tensor_tensor(
                    out=_ap(vt, vt.offset + 65 * 6 * half,
                            [[vt.ap[0][0], n2c], [65, 6], [1, 64]]),
                    in0=ps[0:n2c, 0:384],
                    in1=vb_rep[0:n2c, 384 * half:384 * (half + 1)],
                    op=mybir.AluOpType.add)

            def emit_proj(b, ci, half, ys):
                """proj for batch b, token chunk ci, 384-col half."""
                n2o, n2c = N1C[ci]
                to = N_TOK * b + n2o
                ps = mmps.tile([128, 512], dt.float32, name="mm", tag="mm")
                for k in range(6):
                    nc.tensor.matmul(ps[0:n2c, 0:384],
                                     ao[k][:, to:to + n2c],
                                     wp[k][:, 384 * half:384 * (half + 1)],
                                     start=(k == 0), stop=(k == 5))
                nc.vector.tensor_tensor(
                    out=ys[0:n2c, 384 * half:384 * (half + 1)],
                    in0=ps[0:n2c, 0:384],
                    in1=pb_rep[0:n2c, 384 * half:384 * (half + 1)],
                    op=mybir.AluOpType.add)

            def proj_units(b):
                ys = [ysp.tile([128, DIM], dt.float32, name="ys0", tag="ys0"),
                      ysp.tile([69, DIM], dt.float32, name="ys1", tag="ys1")]
                units = []
                for ci in range(2):
                    for half in range(2):
                        def u(b=b, ci=ci, half=half):
                            ensure_norm(b)
                            emit_proj(b, ci, half, ys[ci])
                        units.append(u)

                    def out_dma(b=b, ci=ci):
                        n2o, n2c = N1C[ci]
                        nc.sync.dma_start(
                            out=y_d[N_TOK * b + n2o:N_TOK * b + n2o + n2c, :],
                            in_=ys[ci][0:n2c, :])
                    units.append(out_dma)
                return units

            # ---------- attention ----------
            SOFF = [0, 256]

            def emit_scores(b, hp):
                """S^T + bias -> exp for head pair hp of batch b.
                One psum tile per head (hi): a psum bank must only ever be
                written by matmuls of a single tile_position mode --
                mixing (0,0) and (64,0) groups in one bank wedges the PE."""
                sph = [mmps.tile([128, 512], dt.float32, name="mm", tag="mm")
                       for _ in range(2)]
                qt = qk_sb[hp]
                kt = qk_sb[6 + hp]
                for hi in range(2):
                    po = 64 * hi
                    for ci, (n2o, n2c) in enumerate(N1C):
                        nc.tensor.matmul(
                            sph[hi][0:n2c, SOFF[ci]:SOFF[ci] + 197],
                            kt[po:po + 64,
                               N_TOK * b + n2o:N_TOK * b + n2o + n2c],
                            qt[po:po + 64, N_TOK * b:N_TOK * (b + 1)],
                            start=True, stop=True)
                ss0 = ss0p.tile([128, 394], dt.float16, name="ss0", tag="ss0")
                ss1 = ss1p.tile([69, 394], dt.float16, name="ss1", tag="ss1")
                ssx = (ss0, ss1)
                bTx = (bT0_sb, bT1_sb)
                for ci, (n2o, n2c) in enumerate(N1C):
                    for hi in range(2):
                        if "bias" in F:
                            nc.vector.tensor_tensor(
                                out=ssx[ci][0:n2c, 197 * hi:197 * (hi + 1)],
                                in0=sph[hi][0:n2c, SOFF[ci]:SOFF[ci] + 197],
                                in1=bTx[ci][hp][0:n2c, 197 * hi:197 * (hi + 1)],
                                op=mybir.AluOpType.add)
                        else:
                            nc.vector.tensor_copy(
                                ssx[ci][0:n2c, 197 * hi:197 * (hi + 1)],
                                sph[hi][0:n2c, SOFF[ci]:SOFF[ci] + 197])
                et0 = et0p.tile([128, 394], dt.bfloat16, name="et0", tag="et0")
                et1 = et1p.tile([69, 394], dt.bfloat16, name="et1", tag="et1")
                if "exp" in F:
                    nc.scalar.activation(out=et0[:, :], in_=ss0[:, :], func=Exp)
                    nc.scalar.activation(out=et1[:, :], in_=ss1[:, :], func=Exp)
                else:
                    nc.vector.tensor_copy(et0[:, :], ss0[:, :])
                    nc.vector.tensor_copy(et1[:, :], ss1[:, :])
                if debug and b == 0 and hp == 0:
                    nc.sync.dma_start(out=dbg_ss0_d[:, :], in_=ss0[:, :])
                    nc.sync.dma_start(out=dbg_ss1_d[:, :], in_=ss1[:, :])
                    nc.sync.dma_start(out=dbg_et0_d[:, :], in_=et0[:, :])
                    nc.sync.dma_start(out=dbg_et1_d[:, :], in_=et1[:, :])
                return et0, et1

            def emit_av(b, hp, et0, et1, rd_t, anums):
                """AV matmuls; stash numerator (bf16) in sbuf and ship the
                denominator row to the batch's DRAM staging buffer."""
                ap_ = avps.tile([128, 512], dt.float32, name="av", tag="av")
                for hi in range(2):
                    h = 2 * hp + hi
                    for ci, (n2o, n2c) in enumerate(N1C):
                        et = (et0, et1)[ci]
                        nc.tensor.matmul(
                            ap_[0:65, 197 * hi:197 * (hi + 1)],
                            v_sb[b][ci][:, 65 * h:65 * (h + 1)],
                            et[0:n2c, 197 * hi:197 * (hi + 1)],
                            start=(ci == 0), stop=(ci == 1))
                if "rt" not in F:
                    for hi in range(2):
                        nc.vector.tensor_copy(
                            ao[hp][64 * hi:64 * (hi + 1),
                                   N_TOK * b:N_TOK * (b + 1)],
                            ap_[0:64, 197 * hi:197 * (hi + 1)])
                    return
                anum = anump.tile([64, 394], dt.bfloat16, name="an", tag="an")
                nc.vector.tensor_copy(anum[0:64, :], ap_[0:64, 0:394])
                rc = rcp.tile([65, 396], dt.float32, name="rc", tag="rc")
                nc.vector.tensor_copy(rc[64:65, 0:394], ap_[64:65, 0:394])
                nc.vector.tensor_copy(rc[64:65, 394:396], ap_[64:65, 0:2])
                nc.scalar.dma_start(out=rd_t[0:1, 396 * hp:396 * (hp + 1)],
                                  in_=rc[64:65, 0:396])
                anums.append((hp, anum))

            def finish_batch(b, rd_t, anums):
                """One wrapped reciprocal for the whole batch's denominators
                (2376 = 99*24 elements) + broadcasts. Returns a closure that
                emits the DVE multiplies -- callers defer it so the DVE
                stream doesn't stall on the DMA chain."""
                rw = rwp.tile([99, 24], dt.float32, name="rw", tag="rw")
                nc.scalar.dma_start(out=rw[0:99, :],
                                    in_=_ap(rd_t, rd_t.offset,
                                            [[24, 99], [1, 24]]))
                rwr = rwp.tile([99, 24], dt.float32, name="rwr", tag="rwr")
                nc.vector.reciprocal(rwr[0:99, :], rw[0:99, :])
                rd2_t = dramp.tile([1, 2376], dt.float32, name="rd2", tag="rd2")
                nc.scalar.dma_start(out=_ap(rd2_t, rd2_t.offset,
                                            [[24, 99], [1, 24]]),
                                    in_=rwr[0:99, :])
                rbs = []
                for hp, anum in anums:
                    rb = rbp.tile([64, 394], dt.float32, name="rb", tag="rb")
                    nc.scalar.dma_start(
                        out=rb[0:64, :],
                        in_=_ap(rd2_t, rd2_t.offset + 396 * hp,
                                [[0, 64], [1, 394]]))
                    if debug and b == 0 and hp == 0:
                        nc.sync.dma_start(out=dbg_rc_d[0:1, :],
                                          in_=_ap(rd2_t, rd2_t.offset,
                                                  [[0, 1], [1, 394]]))
                        nc.sync.dma_start(out=dbg_rb_d[0:64, :],
                                          in_=rb[0:64, :])
                    rbs.append((hp, anum, rb))

                def do_mults(b=b, rbs=rbs):
                    for hp, anum, rb in rbs:
                        for hi in range(2):
                            nc.vector.tensor_tensor(
                                out=ao[hp][64 * hi:64 * (hi + 1),
                                           N_TOK * b:N_TOK * (b + 1)],
                                in0=anum[0:64, 197 * hi:197 * (hi + 1)],
                                in1=rb[0:64, 197 * hi:197 * (hi + 1)],
                                op=mybir.AluOpType.mult)
                return do_mults

            # ---------- main schedule ----------
            # prologue: qkv for chunk 0 (batches 0,1), dense
            for m in range(12):
                emit_qk_m(0, m)
            for b in (0, 1):
                for ci in range(2):
                    for half in range(2):
                        emit_v(b, ci, half)

            # per chunk: attention for its 2 batches, with next-chunk qkv and
            # previous-batch proj emitted as PE filler between pair stages.
            pending_norm = {}

            def ensure_norm(b):
                f = pending_norm.pop(b, None)
                if f is not None:
                    f()

            for c in range(nchunks):
                filler = []
                if c + 1 < nchunks:
                    filler += [lambda m=m, c=c: emit_qk_m(c + 1, m)
                               for m in range(12)]
                    for b in (2 * (c + 1), 2 * (c + 1) + 1):
                        filler += [lambda b=b, ci=ci, half=half:
                                   emit_v(b, ci, half)
                                   for ci in range(2) for half in range(2)]
                if c >= 1 and "proj" in F:
                    filler += proj_units(2 * (c - 1))
                    filler += proj_units(2 * (c - 1) + 1)
                fit = iter(filler)

                def fill(n=1):
                    if "fill" not in F:
                        return
                    for _ in range(n):
                        u = next(fit, None)
                        if u is not None:
                            u()

                for b in (2 * c, 2 * c + 1):
                    if "scores" not in F:
                        continue
                    rd_t = dramp.tile([1, 2376], dt.float32, name="rd",
                                      tag="rd")
                    anums = []
                    for bb in [k for k in pending_norm if k < b]:
                        ensure_norm(bb)   # prev batch's normalize mults
                    pend = []  # (hp, et0, et1) awaiting AV
                    for hp in range(6):
                        ets = emit_scores(b, hp)
                        fill(2)
                        pend.append((hp, ets))
                        if "av" in F and len(pend) >= 2:
                            php, (e0, e1) = pend.pop(0)
                            emit_av(b, php, e0, e1, rd_t, anums)
                            fill(1)
                    if "av" in F:
                        for php, (e0, e1) in pend:
                            emit_av(b, php, e0, e1, rd_t, anums)
                            fill(1)
                        if "rt" in F:
                            pending_norm[b] = finish_batch(b, rd_t, anums)
                # drain leftover filler before next chunk's attention
                for u in fit:
                    u()

            # epilogue: proj for the last two batches
            if "proj" in F:
                for u in proj_units(nb - 2):
                    u()
                for u in proj_units(nb - 1):
                    u()

            if debug:
                for m in range(12):
                    nc.sync.dma_start(out=dbg_qk_d[128 * m:128 * (m + 1), :],
                                      in_=qk_sb[m][:, :])
                for ci in range(2):
                    n2c = N1C[ci][1]
                    nc.sync.dma_start(
                        out=dbg_v_d[128 * ci:128 * ci + n2c, :],
                        in_=v_sb[0][ci][:, :])
                for m in range(6):
                    nc.sync.dma_start(out=dbg_ao_d[128 * m:128 * (m + 1), :],
                                      in_=ao[m][:, :])

    nc.compile()
    return nc


def _marshal(x, qkv_w, q_bias, v_bias, rpb_table, proj_w, proj_b, rel_index):
    B = x.shape[0]
    ncore = 8
    bpc = B // ncore

    wqkT = np.ascontiguousarray(qkv_w[0:2 * DIM, :].T.astype(np.float32))
    wqkT[:, 0:DIM] *= SCALE
    wqkT = wqkT.astype(BF16)
    wvT = np.ascontiguousarray(qkv_w[2 * DIM:3 * DIM, :].T.astype(BF16))
    wpT = np.ascontiguousarray(proj_w.T.astype(BF16))
    qb2 = np.ascontiguousarray(
        (q_bias.astype(np.float32) * SCALE).reshape(6, 128).T)

    # full transposed bias, pair-packed: bias[h][n1, n2] -> biasT[h][n2, n1]
    bias = rpb_table[np.asarray(rel_index).reshape(-1)].reshape(
        N_TOK, N_TOK, NH).astype(np.float32)  # [n1, n2, h]
    bT0 = np.zeros((6 * 128, 394), dtype=BF16)
    bT1 = np.zeros((6 * 69, 394), dtype=BF16)
    for hp in range(6):
        for hi in range(2):
            bt = bias[:, :, 2 * hp + hi].T  # [n2, n1]
            bT0[128 * hp:128 * (hp + 1), 197 * hi:197 * (hi + 1)] = bt[0:128, :]
            bT1[69 * hp:69 * (hp + 1), 197 * hi:197 * (hi + 1)] = bt[128:197, :]

    shared = {"wqkT": wqkT, "wvT": wvT, "wpT": wpT, "qb2": qb2,
              "vb": np.ascontiguousarray(v_bias.astype(np.float32)),
              "pb": np.ascontiguousarray(proj_b.astype(np.float32)),
              "bT0": bT0, "bT1": bT1}
    x2 = np.asarray(x, dtype=np.float32).reshape(B, N_TOK, DIM)
    in_maps = []
    for c in range(ncore):
        xTb = np.ascontiguousarray(
            x2[c * bpc:(c + 1) * bpc].reshape(bpc * N_TOK, DIM).T.astype(BF16))
        m = dict(shared)
        m["xTb"] = xTb
        in_maps.append(m)
    return in_maps, bpc


last_exec_time_ns = None
last_results = None


def _install_ntff_hook():
    """Provide antenv.axon_hooks + register the ctypes NTFF hook (the agent
    image's antenv lacks axon_hooks, so trn_boot degraded silently)."""
    import types
    import contextlib
    import ctypes

    try:
        from antenv.axon_hooks import get_axon_ntff_profile_hook
        if get_axon_ntff_profile_hook() is not None:
            return
    except ImportError:
        import antenv
        mod = types.ModuleType("antenv.axon_hooks")
        mod._hook = None

        def set_axon_ntff_profile_hook(h):
            mod._hook = h

        def get_axon_ntff_profile_hook():
            return mod._hook

        mod.set_axon_ntff_profile_hook = set_axon_ntff_profile_hook
        mod.get_axon_ntff_profile_hook = get_axon_ntff_profile_hook
        sys.modules["antenv.axon_hooks"] = mod
        antenv.axon_hooks = mod

    so_path = "/opt/axon/libaxon_pjrt.so"
    lib = ctypes.CDLL(so_path)
    if not hasattr(lib, "axon_start_nrt_profile"):
        return
    lib.axon_start_nrt_profile.argtypes = [ctypes.POINTER(ctypes.c_int64),
                                           ctypes.c_size_t]
    lib.axon_start_nrt_profile.restype = ctypes.c_int64
    lib.axon_stop_nrt_profile.argtypes = [ctypes.c_char_p]
    lib.axon_stop_nrt_profile.restype = ctypes.c_int64

    @contextlib.contextmanager
    def _hook(output_dir, device_ids):
        import jax
        jax.devices()
        if device_ids:
            ids = (ctypes.c_int64 * len(device_ids))(*device_ids)
            rc = lib.axon_start_nrt_profile(ids, len(device_ids))
        else:
            rc = lib.axon_start_nrt_profile(None, 0)
        if rc != 0:
            raise RuntimeError(f"axon_start_nrt_profile rc={rc}")
        try:
            yield
        finally:
            n = lib.axon_stop_nrt_profile(str(output_dir).encode())
            print(f"ntff profile: {n} file(s) -> {output_dir}", file=sys.stderr)

    from antenv.axon_hooks import set_axon_ntff_profile_hook
    set_axon_ntff_profile_hook(_hook)


def kernel(x, qkv_w, q_bias, v_bias, rpb_table, proj_w, proj_b, rel_index):
    global last_exec_time_ns, last_results
    import os
    if os.environ.get("KERNEL_TRACE"):
        _install_ntff_hook()
    from concourse.bass_utils import run_bass_kernel_spmd

    x = np.asarray(x, dtype=np.float32)
    qkv_w = np.asarray(qkv_w, dtype=np.float32)
    q_bias = np.asarray(q_bias, dtype=np.float32)
    v_bias = np.asarray(v_bias, dtype=np.float32)
    rpb_table = np.asarray(rpb_table, dtype=np.float32)
    proj_w = np.asarray(proj_w, dtype=np.float32)
    proj_b = np.asarray(proj_b, dtype=np.float32)

    B = x.shape[0]
    bpc = B // 8
    if 'nc' not in _cache:
        _cache['nc'] = build_program(bpc)
    nc = _cache['nc']

    in_maps, bpc = _marshal(x, qkv_w, q_bias, v_bias, rpb_table,
                            proj_w, proj_b, rel_index)
    res = run_bass_kernel_spmd(nc, in_maps, core_ids=list(range(8)),
                               trace=bool(os.environ.get("KERNEL_TRACE")))
    last_exec_time_ns = res.exec_time_ns
    last_results = res
    ys = [res.results[c]["y"].reshape(bpc, N_TOK, DIM) for c in range(8)]
    return np.concatenate(ys, axis=0).astype(np.float32)


# revision 24
# speedup vs baseline: 1.0723x; 1.0723x over previous
"""TRN2 Bass kernel for BEiT-style attention (nn_Attention_27771258536423).

Strategy: data-parallel over batch across 8 NeuronCores (8 batches/core).
Per core (all matmuls bf16, psum f32):
  - rel-pos bias precomputed on host, shipped TRANSPOSED and pair-packed.
  - qkv: q,k channel-major [ch, tok] bf16 (q pre-scaled+biased via folded
    weights); v token-major [tok, 65*12] bf16 with a ones column per head
    (denominator rides along in the AV matmul).
  - attention per (batch, head-pair): S^T = k^T q directly (no transposes
    anywhere: exp(S^T + bias^T) IS E^T, the AV moving operand).
    AV: out[65, 197] = V_aug^T E^T -> rows 0:64 = attn_out^T (channel-major),
    row 64 = softmax denominator. reciprocal (DVE) -> partition_broadcast
    (gpsimd) -> multiply (DVE) into attn_outT bf16.
  - proj per batch, interleaved into the attention stream as PE filler
    together with next-chunk qkv matmuls (covers softmax latency).
"""
import sys

sys.path.insert(0, '/opt/trn_rl_repo')

import numpy as np
import ml_dtypes

import concourse.bass as bass
import concourse.mybir as mybir
import concourse.tile as tile
from concourse import bacc

dt = mybir.dt
BF16 = ml_dtypes.bfloat16

DIM = 768
NH = 12
HD = 64
N_TOK = 197
SCALE = HD ** (-0.5)
CHUNK = 2 * N_TOK          # 394 tokens = 2 batches per qkv chunk
N1C = [(0, 128), (128, 69)]  # token chunks within one batch (n2 chunks)

_cache = {}


def _ap(t, offset, ap):
    return bass.AP(tensor=t.tensor if hasattr(t, 'tensor') else t,
                   offset=offset, ap=ap)


def build_program(nb, debug=False, feats="scores,bias,exp,av,rt,proj,fill"):
    F = set(feats.split(",")) if feats else set()
    """nb = batches per core (8). Returns compiled Bacc."""
    assert nb % 2 == 0
    ntok = nb * N_TOK
    nchunks = nb // 2

    nc = bacc.Bacc(None)
    if debug:
        dbg_qk_d = nc.dram_tensor("dbg_qk", [12 * 128, ntok], dt.bfloat16,
                                  kind="ExternalOutput")
        dbg_v_d = nc.dram_tensor("dbg_v", [2 * 128, 65 * NH], dt.bfloat16,
                                 kind="ExternalOutput")
        dbg_ss0_d = nc.dram_tensor("dbg_ss0", [128, 394], dt.float32,
                                   kind="ExternalOutput")
        dbg_ss1_d = nc.dram_tensor("dbg_ss1", [69, 394], dt.float32,
                                   kind="ExternalOutput")
        dbg_et0_d = nc.dram_tensor("dbg_et0", [128, 394], dt.bfloat16,
                                   kind="ExternalOutput")
        dbg_et1_d = nc.dram_tensor("dbg_et1", [69, 394], dt.bfloat16,
                                   kind="ExternalOutput")
        dbg_rc_d = nc.dram_tensor("dbg_rc", [1, 394], dt.float32,
                                  kind="ExternalOutput")
        dbg_rb_d = nc.dram_tensor("dbg_rb", [64, 394], dt.float32,
                                  kind="ExternalOutput")
        dbg_ao_d = nc.dram_tensor("dbg_ao", [6 * 128, ntok], dt.bfloat16,
                                  kind="ExternalOutput")

    xTb_d = nc.dram_tensor("xTb", [DIM, ntok], dt.bfloat16, kind="ExternalInput")
    wqkT_d = nc.dram_tensor("wqkT", [DIM, 2 * DIM], dt.bfloat16, kind="ExternalInput")
    wvT_d = nc.dram_tensor("wvT", [DIM, DIM], dt.bfloat16, kind="ExternalInput")
    wpT_d = nc.dram_tensor("wpT", [DIM, DIM], dt.bfloat16, kind="ExternalInput")
    qb2_d = nc.dram_tensor("qb2", [128, 6], dt.float32, kind="ExternalInput")
    vb_d = nc.dram_tensor("vb", [DIM], dt.float32, kind="ExternalInput")
    pb_d = nc.dram_tensor("pb", [DIM], dt.float32, kind="ExternalInput")
    bT0_d = nc.dram_tensor("bT0", [6 * 128, 394], dt.bfloat16, kind="ExternalInput")
    bT1_d = nc.dram_tensor("bT1", [6 * 69, 394], dt.bfloat16, kind="ExternalInput")
    y_d = nc.dram_tensor("y", [ntok, DIM], dt.float32, kind="ExternalOutput")

    Exp = mybir.ActivationFunctionType.Exp

    with tile.TileContext(nc) as tc:
        import contextlib
        with contextlib.ExitStack() as stk:
            consts = stk.enter_context(tc.tile_pool(name="consts", bufs=1))
            wpool = stk.enter_context(tc.tile_pool(name="wpool", bufs=1))
            xp = stk.enter_context(tc.tile_pool(name="xp", bufs=1))
            qkp = stk.enter_context(tc.tile_pool(name="qkp", bufs=1))
            vp = stk.enter_context(tc.tile_pool(name="vp", bufs=1))
            aop = stk.enter_context(tc.tile_pool(name="aop", bufs=1))
            ss0p = stk.enter_context(tc.tile_pool(name="ss0p", bufs=3))
            ss1p = stk.enter_context(tc.tile_pool(name="ss1p", bufs=3))
            et0p = stk.enter_context(tc.tile_pool(name="et0p", bufs=3))
            et1p = stk.enter_context(tc.tile_pool(name="et1p", bufs=3))
            rcp = stk.enter_context(tc.tile_pool(name="rcp", bufs=4))
            rbp = stk.enter_context(tc.tile_pool(name="rbp", bufs=8))
            anump = stk.enter_context(tc.tile_pool(name="anump", bufs=8))
            rwp = stk.enter_context(tc.tile_pool(name="rwp", bufs=2))
            ysp = stk.enter_context(tc.tile_pool(name="ysp", bufs=3))
            mmps = stk.enter_context(tc.tile_pool(name="mmps", bufs=5, space="PSUM"))
            avps = stk.enter_context(tc.tile_pool(name="avps", bufs=3, space="PSUM"))
            dramp = stk.enter_context(tc.tile_pool(name="dramp", bufs=2, space="DRAM"))

            # ---------- constant / weight DMAs ----------
            # x chunk 0 + q-weight tiles first (first matmuls need them)
            xb = [xp.tile([128, ntok], dt.bfloat16, name=f"xb{k}", tag=f"xb{k}")
                  for k in range(6)]
            for k in range(6):
                nc.sync.dma_start(out=xb[k][:, 0:CHUNK],
                                  in_=xTb_d[128 * k:128 * (k + 1), 0:CHUNK])
            wqk = [wpool.tile([128, 2 * DIM], dt.bfloat16, name=f"wqk{k}",
                              tag=f"wqk{k}") for k in range(6)]
            for k in range(6):   # q columns first
                nc.sync.dma_start(out=wqk[k][:, 0:DIM],
                                  in_=wqkT_d[128 * k:128 * (k + 1), 0:DIM])
            qb2_sb = consts.tile([128, 6], dt.float32, name="qb2", tag="qb2")
            nc.sync.dma_start(out=qb2_sb[:, :], in_=qb2_d[:, :])
            for k in range(6):   # k columns
                nc.sync.dma_start(out=wqk[k][:, DIM:2 * DIM],
                                  in_=wqkT_d[128 * k:128 * (k + 1), DIM:2 * DIM])
            wv = [wpool.tile([128, DIM], dt.bfloat16, name=f"wv{k}", tag=f"wv{k}")
                  for k in range(6)]
            for k in range(6):
                nc.sync.dma_start(out=wv[k][:, :],
                                  in_=wvT_d[128 * k:128 * (k + 1), :])
            vb_rep = consts.tile([128, DIM], dt.float32, name="vbrep", tag="vbrep")
            nc.sync.dma_start(out=vb_rep[:, :],
                              in_=_ap(vb_d, 0, [[0, 128], [1, DIM]]))
            # bias tiles (needed at attention time)
            bT0_sb = []
            bT1_sb = []
            for hp in range(6):
                b0 = consts.tile([128, 394], dt.bfloat16, name=f"bT0_{hp}",
                                 tag=f"bT0_{hp}")
                nc.sync.dma_start(out=b0[:, :],
                                  in_=bT0_d[128 * hp:128 * (hp + 1), :])
                bT0_sb.append(b0)
                b1 = consts.tile([69, 394], dt.bfloat16, name=f"bT1_{hp}",
                                 tag=f"bT1_{hp}")
                nc.sync.dma_start(out=b1[:, :],
                                  in_=bT1_d[69 * hp:69 * (hp + 1), :])
                bT1_sb.append(b1)
            # remaining x chunks
            for c in range(1, nchunks):
                for k in range(6):
                    nc.sync.dma_start(
                        out=xb[k][:, CHUNK * c:CHUNK * (c + 1)],
                        in_=xTb_d[128 * k:128 * (k + 1), CHUNK * c:CHUNK * (c + 1)])
            wp = [wpool.tile([128, DIM], dt.bfloat16, name=f"wp{k}", tag=f"wp{k}")
                  for k in range(6)]
            for k in range(6):
                nc.sync.dma_start(out=wp[k][:, :],
                                  in_=wpT_d[128 * k:128 * (k + 1), :])
            pb_rep = consts.tile([128, DIM], dt.float32, name="pbrep", tag="pbrep")
            nc.sync.dma_start(out=pb_rep[:, :],
                              in_=_ap(pb_d, 0, [[0, 128], [1, DIM]]))

            # ---------- persistent sbuf tiles ----------
            qk_sb = [qkp.tile([128, ntok], dt.bfloat16, name=f"qk{m}", tag=f"qk{m}")
                     for m in range(12)]  # 0-5 q (scaled+biased), 6-11 k
            # v: token-major, 65 cols per head (64 v + shared ones col)
            v_sb = [[vp.tile([n2c, 65 * NH], dt.bfloat16, name=f"v{b}_{ci}",
                             tag=f"v{b}_{ci}")
                     for ci, (n2o, n2c) in enumerate(N1C)] for b in range(nb)]
            ao = [aop.tile([128, ntok], dt.bfloat16, name=f"ao{m}", tag=f"ao{m}")
                  for m in range(6)]  # attn_out^T, head pair per tile

            # ---------- work-unit emitters ----------
            def emit_qk_m(c, m):
                """q/k projection for block m, token chunk c."""
                no = CHUNK * c
                ps = mmps.tile([128, 512], dt.float32, name="mm", tag="mm")
                col = 128 * m if m < 6 else DIM + 128 * (m - 6)
                for k in range(6):
                    nc.tensor.matmul(ps[:, 0:CHUNK],
                                     wqk[k][:, col:col + 128],
                                     xb[k][:, no:no + CHUNK],
                                     start=(k == 0), stop=(k == 5))
                if m < 6:
                    nc.vector.tensor_scalar(
                        out=qk_sb[m][:, no:no + CHUNK], in0=ps[:, 0:CHUNK],
                        scalar1=qb2_sb[:, m:m + 1], scalar2=None,
                        op0=mybir.AluOpType.add)
                else:
                    nc.vector.tensor_copy(qk_sb[m][:, no:no + CHUNK],
                                          ps[:, 0:CHUNK])

            def emit_v(b, ci, half):
                """v projection for batch b, token chunk ci, 384-col half."""
                n2o, n2c = N1C[ci]
                vt = v_sb[b][ci]
                if half == 0:  # ones columns once per tile
                    nc.vector.memset(
                        _ap(vt, vt.offset + 64,
                            [[vt.ap[0][0], n2c], [65, NH]]), 1.0)
                ps = mmps.tile([128, 512], dt.float32, name="mm", tag="mm")
                for k in range(6):
                    nc.tensor.matmul(
                        ps[0:n2c, 0:384],
                        xb[k][:, N_TOK * b + n2o:N_TOK * b + n2o + n2c],
                        wv[k][:, 384 * half:384 * (half + 1)],
                        start=(k == 0), stop=(k == 5))
                nc.vector.tensor_tensor(
                    out=_ap(vt, vt.offset + 65 * 6 * half,
                            [[vt.ap[0][0], n2c], [65, 6], [1, 64]]),
                    in0=ps[0:n2c, 0:384],
                    in1=vb_rep[0:n2c, 384 * half:384 * (half + 1)],
                    op=mybir.AluOpType.add)

            def emit_proj(b, ci, half, ys):
                """proj for batch b, token chunk ci, 384-col half."""
                n2o, n2c = N1C[ci]
                to = N_TOK * b + n2o
                ps = mmps.tile([128, 512], dt.float32, name="mm", tag="mm")
                for k in range(6):
                    nc.tensor.matmul(ps[0:n2c, 0:384],
                                     ao[k][:, to:to + n2c],
                                     wp[k][:, 384 * half:384 * (half + 1)],
                                     start=(k == 0), stop=(k == 5))
                nc.vector.tensor_tensor(
                    out=ys[0:n2c, 384 * half:384 * (half + 1)],
                    in0=ps[0:n2c, 0:384],
                    in1=pb_rep[0:n2c, 384 * half:384 * (half + 1)],
                    op=mybir.AluOpType.add)

            def proj_units(b):
                ys = [ysp.tile([128, DIM], dt.float32, name="ys0", tag="ys0"),
                      ysp.tile([69, DIM], dt.float32, name="ys1", tag="ys1")]
                units = []
                for ci in range(2):
                    for half in range(2):
                        def u(b=b, ci=ci, half=half):
                            ensure_norm(b)
                            emit_proj(b, ci, half, ys[ci])
                        units.append(u)

                    def out_dma(b=b, ci=ci):
                        n2o, n2c = N1C[ci]
                        nc.sync.dma_start(
                            out=y_d[N_TOK * b + n2o:N_TOK * b + n2o + n2c, :],
                            in_=ys[ci][0:n2c, :])
                    units.append(out_dma)
                return units

            # ---------- attention ----------
            SOFF = [0, 256]

            def emit_scores(b, hp):
                """S^T + bias -> exp for head pair hp of batch b.
                One psum tile per head (hi): a psum bank must only ever be
                written by matmuls of a single tile_position mode --
                mixing (0,0) and (64,0) groups in one bank wedges the PE."""
                sph = [mmps.tile([128, 512], dt.float32, name="mm", tag="mm")
                       for _ in range(2)]
                qt = qk_sb[hp]
                kt = qk_sb[6 + hp]
                for hi in range(2):
                    po = 64 * hi
                    for ci, (n2o, n2c) in enumerate(N1C):
                        nc.tensor.matmul(
                            sph[hi][0:n2c, SOFF[ci]:SOFF[ci] + 197],
                            kt[po:po + 64,
                               N_TOK * b + n2o:N_TOK * b + n2o + n2c],
                            qt[po:po + 64, N_TOK * b:N_TOK * (b + 1)],
                            start=True, stop=True)
                ss0 = ss0p.tile([128, 394], dt.float16, name="ss0", tag="ss0")
                ss1 = ss1p.tile([69, 394], dt.float16, name="ss1", tag="ss1")
                ssx = (ss0, ss1)
                bTx = (bT0_sb, bT1_sb)
                for ci, (n2o, n2c) in enumerate(N1C):
                    for hi in range(2):
                        if "bias" in F:
                            nc.vector.tensor_tensor(
                                out=ssx[ci][0:n2c, 197 * hi:197 * (hi + 1)],
                                in0=sph[hi][0:n2c, SOFF[ci]:SOFF[ci] + 197],
                                in1=bTx[ci][hp][0:n2c, 197 * hi:197 * (hi + 1)],
                                op=mybir.AluOpType.add)
                        else:
                            nc.vector.tensor_copy(
                                ssx[ci][0:n2c, 197 * hi:197 * (hi + 1)],
                                sph[hi][0:n2c, SOFF[ci]:SOFF[ci] + 197])
                et0 = et0p.tile([128, 394], dt.bfloat16, name="et0", tag="et0")
                et1 = et1p.tile([69, 394], dt.bfloat16, name="et1", tag="et1")
                if "exp" in F:
                    nc.scalar.activation(out=et0[:, :], in_=ss0[:, :], func=Exp)
                    nc.scalar.activation(out=et1[:, :], in_=ss1[:, :], func=Exp)
                else:
                    nc.vector.tensor_copy(et0[:, :], ss0[:, :])
                    nc.vector.tensor_copy(et1[:, :], ss1[:, :])
                if debug and b == 0 and hp == 0:
                    nc.sync.dma_start(out=dbg_ss0_d[:, :], in_=ss0[:, :])
                    nc.sync.dma_start(out=dbg_ss1_d[:, :], in_=ss1[:, :])
                    nc.sync.dma_start(out=dbg_et0_d[:, :], in_=et0[:, :])
                    nc.sync.dma_start(out=dbg_et1_d[:, :], in_=et1[:, :])
                return et0, et1

            def emit_av(b, hp, et0, et1, rd_t, anums):
                """AV matmuls; stash numerator (bf16) in sbuf and ship the
                denominator row to the batch's DRAM staging buffer."""
                ap_ = avps.tile([128, 512], dt.float32, name="av", tag="av")
                for hi in range(2):
                    h = 2 * hp + hi
                    for ci, (n2o, n2c) in enumerate(N1C):
                        et = (et0, et1)[ci]
                        nc.tensor.matmul(
                            ap_[0:65, 197 * hi:197 * (hi + 1)],
                            v_sb[b][ci][:, 65 * h:65 * (h + 1)],
                            et[0:n2c, 197 * hi:197 * (hi + 1)],
                            start=(ci == 0), stop=(ci == 1))
                if "rt" not in F:
                    for hi in range(2):
                        nc.vector.tensor_copy(
                            ao[hp][64 * hi:64 * (hi + 1),
                                   N_TOK * b:N_TOK * (b + 1)],
                            ap_[0:64, 197 * hi:197 * (hi + 1)])
                    return
                anum = anump.tile([64, 394], dt.bfloat16, name="an", tag="an")
                nc.vector.tensor_copy(anum[0:64, :], ap_[0:64, 0:394])
                rc = rcp.tile([65, 396], dt.float32, name="rc", tag="rc")
                nc.vector.tensor_copy(rc[64:65, 0:394], ap_[64:65, 0:394])
                nc.vector.tensor_copy(rc[64:65, 394:396], ap_[64:65, 0:2])
                nc.scalar.dma_start(out=rd_t[0:1, 396 * hp:396 * (hp + 1)],
                                  in_=rc[64:65, 0:396])
                anums.append((hp, anum))

            def finish_batch(b, rd_t, anums):
                """One wrapped reciprocal for the whole batch's denominators
                (2376 = 99*24 elements) + broadcasts. Returns a closure that
                emits the DVE multiplies -- callers defer it so the DVE
                stream doesn't stall on the DMA chain."""
                rw = rwp.tile([99, 24], dt.float32, name="rw", tag="rw")
                nc.scalar.dma_start(out=rw[0:99, :],
                                    in_=_ap(rd_t, rd_t.offset,
                                            [[24, 99], [1, 24]]))
                rwr = rwp.tile([99, 24], dt.float32, name="rwr", tag="rwr")
                nc.vector.reciprocal(rwr[0:99, :], rw[0:99, :])
                rd2_t = dramp.tile([1, 2376], dt.float32, name="rd2", tag="rd2")
                nc.scalar.dma_start(out=_ap(rd2_t, rd2_t.offset,
                                            [[24, 99], [1, 24]]),
                                    in_=rwr[0:99, :])
                rbs = []
                for hp, anum in anums:
                    rb = rbp.tile([64, 394], dt.float32, name="rb", tag="rb")
                    nc.scalar.dma_start(
                        out=rb[0:64, :],
                        in_=_ap(rd2_t, rd2_t.offset + 396 * hp,
                                [[0, 64], [1, 394]]))
                    if debug and b == 0 and hp == 0:
                        nc.sync.dma_start(out=dbg_rc_d[0:1, :],
                                          in_=_ap(rd2_t, rd2_t.offset,
                                                  [[0, 1], [1, 394]]))
                        nc.sync.dma_start(out=dbg_rb_d[0:64, :],
                                          in_=rb[0:64, :])
                    rbs.append((hp, anum, rb))

                # last two batches: DVE (short tail); earlier: gpsimd
                # (parallel engine, keeps DVE free mid-stream)
                eng = nc.vector if b >= nb - 2 else nc.gpsimd
                units = []
                for hp, anum, rb in rbs:
                    for hi in range(2):
                        def mu(hp=hp, anum=anum, rb=rb, hi=hi, eng=eng, b=b):
                            eng.tensor_tensor(
                                out=ao[hp][64 * hi:64 * (hi + 1),
                                           N_TOK * b:N_TOK * (b + 1)],
                                in0=anum[0:64, 197 * hi:197 * (hi + 1)],
                                in1=rb[0:64, 197 * hi:197 * (hi + 1)],
                                op=mybir.AluOpType.mult)
                        units.append(mu)

                def do_mults(units=units):
                    for u in units:
                        u()
                return do_mults

            # ---------- main schedule ----------
            # prologue: qkv for chunk 0 (batches 0,1), dense
            for m in range(12):
                emit_qk_m(0, m)
            for b in (0, 1):
                for ci in range(2):
                    for half in range(2):
                        emit_v(b, ci, half)

            # per chunk: attention for its 2 batches, with next-chunk qkv and
            # previous-batch proj emitted as PE filler between pair stages.
            pending_norm = {}

            def ensure_norm(b):
                f = pending_norm.pop(b, None)
                if f is not None:
                    f()

            for c in range(nchunks):
                filler = []
                if c + 1 < nchunks:
                    filler += [lambda m=m, c=c: emit_qk_m(c + 1, m)
                               for m in range(12)]
                    for b in (2 * (c + 1), 2 * (c + 1) + 1):
                        filler += [lambda b=b, ci=ci, half=half:
                                   emit_v(b, ci, half)
                                   for ci in range(2) for half in range(2)]
                if c >= 1 and "proj" in F:
                    filler += proj_units(2 * (c - 1))
                    filler += proj_units(2 * (c - 1) + 1)
                fit = iter(filler)

                def fill(n=1):
                    if "fill" not in F:
                        return
                    for _ in range(n):
                        u = next(fit, None)
                        if u is not None:
                            u()

                for b in (2 * c, 2 * c + 1):
                    if "scores" not in F:
                        continue
                    rd_t = dramp.tile([1, 2376], dt.float32, name="rd",
                                      tag="rd")
                    anums = []
                    for bb in [k for k in pending_norm if k < b]:
                        ensure_norm(bb)   # prev batch's normalize mults
                    pend = []  # (hp, et0, et1) awaiting AV
                    for hp in range(6):
                        ets = emit_scores(b, hp)
                        fill(2)
                        pend.append((hp, ets))
                        if "av" in F and len(pend) >= 2:
                            php, (e0, e1) = pend.pop(0)
                            emit_av(b, php, e0, e1, rd_t, anums)
                            fill(1)
                    if "av" in F:
                        for php, (e0, e1) in pend:
                            emit_av(b, php, e0, e1, rd_t, anums)
                            fill(1)
                        if "rt" in F:
                            pending_norm[b] = finish_batch(b, rd_t, anums)
                # drain leftover filler before next chunk's attention
                for u in fit:
                    u()

            # epilogue: proj for the last two batches
            if "proj" in F:
                for u in proj_units(nb - 2):
                    u()
                for u in proj_units(nb - 1):
                    u()

            if debug:
                for m in range(12):
                    nc.sync.dma_start(out=dbg_qk_d[128 * m:128 * (m + 1), :],
                                      in_=qk_sb[m][:, :])
                for ci in range(2):
                    n2c = N1C[ci][1]
                    nc.sync.dma_start(
                        out=dbg_v_d[128 * ci:128 * ci + n2c, :],
                        in_=v_sb[0][ci][:, :])
                for m in range(6):
                    nc.sync.dma_start(out=dbg_ao_d[128 * m:128 * (m + 1), :],
                                      in_=ao[m][:, :])

    nc.compile()
    return nc


def _marshal(x, qkv_w, q_bias, v_bias, rpb_table, proj_w, proj_b, rel_index):
    B = x.shape[0]
    ncore = 8
    bpc = B // ncore

    wqkT = np.ascontiguousarray(qkv_w[0:2 * DIM, :].T.astype(np.float32))
    wqkT[:, 0:DIM] *= SCALE
    wqkT = wqkT.astype(BF16)
    wvT = np.ascontiguousarray(qkv_w[2 * DIM:3 * DIM, :].T.astype(BF16))
    wpT = np.ascontiguousarray(proj_w.T.astype(BF16))
    qb2 = np.ascontiguousarray(
        (q_bias.astype(np.float32) * SCALE).reshape(6, 128).T)

    # full transposed bias, pair-packed: bias[h][n1, n2] -> biasT[h][n2, n1]
    bias = rpb_table[np.asarray(rel_index).reshape(-1)].reshape(
        N_TOK, N_TOK, NH).astype(np.float32)  # [n1, n2, h]
    bT0 = np.zeros((6 * 128, 394), dtype=BF16)
    bT1 = np.zeros((6 * 69, 394), dtype=BF16)
    for hp in range(6):
        for hi in range(2):
            bt = bias[:, :, 2 * hp + hi].T  # [n2, n1]
            bT0[128 * hp:128 * (hp + 1), 197 * hi:197 * (hi + 1)] = bt[0:128, :]
            bT1[69 * hp:69 * (hp + 1), 197 * hi:197 * (hi + 1)] = bt[128:197, :]

    shared = {"wqkT": wqkT, "wvT": wvT, "wpT": wpT, "qb2": qb2,
              "vb": np.ascontiguousarray(v_bias.astype(np.float32)),
              "pb": np.ascontiguousarray(proj_b.astype(np.float32)),
              "bT0": bT0, "bT1": bT1}
    x2 = np.asarray(x, dtype=np.float32).reshape(B, N_TOK, DIM)
    in_maps = []
    for c in range(ncore):
        xTb = np.ascontiguousarray(
            x2[c * bpc:(c + 1) * bpc].reshape(bpc * N_TOK, DIM).T.astype(BF16))
        m = dict(shared)
        m["xTb"] = xTb
        in_maps.append(m)
    return in_maps, bpc


last_exec_time_ns = None
last_results = None


def _install_ntff_hook():
    """Provide antenv.axon_hooks + register the ctypes NTFF hook (the agent
    image's antenv lacks axon_hooks, so trn_boot degraded silently)."""
    import types
    import contextlib
    import ctypes

    try:
        from antenv.axon_hooks import get_axon_ntff_profile_hook
        if get_axon_ntff_profile_hook() is not None:
            return
    except ImportError:
        import antenv
        mod = types.ModuleType("antenv.axon_hooks")
        mod._hook = None

        def set_axon_ntff_profile_hook(h):
            mod._hook = h

        def get_axon_ntff_profile_hook():
            return mod._hook

        mod.set_axon_ntff_profile_hook = set_axon_ntff_profile_hook
        mod.get_axon_ntff_profile_hook = get_axon_ntff_profile_hook
        sys.modules["antenv.axon_hooks"] = mod
        antenv.axon_hooks = mod

    so_path = "/opt/axon/libaxon_pjrt.so"
    lib = ctypes.CDLL(so_path)
    if not hasattr(lib, "axon_start_nrt_profile"):
        return
    lib.axon_start_nrt_profile.argtypes = [ctypes.POINTER(ctypes.c_int64),
                                           ctypes.c_size_t]
    lib.axon_start_nrt_profile.restype = ctypes.c_int64
    lib.axon_stop_nrt_profile.argtypes = [ctypes.c_char_p]
    lib.axon_stop_nrt_profile.restype = ctypes.c_int64

    @contextlib.contextmanager
    def _hook(output_dir, device_ids):
        import jax
        jax.devices()
        if device_ids:
            ids = (ctypes.c_int64 * len(device_ids))(*device_ids)
            rc = lib.axon_start_nrt_profile(ids, len(device_ids))
        else:
            rc = lib.axon_start_nrt_profile(None, 0)
        if rc != 0:
            raise RuntimeError(f"axon_start_nrt_profile rc={rc}")
        try:
            yield
        finally:
            n = lib.axon_stop_nrt_profile(str(output_dir).encode())
            print(f"ntff profile: {n} file(s) -> {output_dir}", file=sys.stderr)

    from antenv.axon_hooks import set_axon_ntff_profile_hook
    set_axon_ntff_profile_hook(_hook)


def kernel(x, qkv_w, q_bias, v_bias, rpb_table, proj_w, proj_b, rel_index):
    global last_exec_time_ns, last_results
    import os
    if os.environ.get("KERNEL_TRACE"):
        _install_ntff_hook()
    from concourse.bass_utils import run_bass_kernel_spmd

    x = np.asarray(x, dtype=np.float32)
    qkv_w = np.asarray(qkv_w, dtype=np.float32)
    q_bias = np.asarray(q_bias, dtype=np.float32)
    v_bias = np.asarray(v_bias, dtype=np.float32)
    rpb_table = np.asarray(rpb_table, dtype=np.float32)
    proj_w = np.asarray(proj_w, dtype=np.float32)
    proj_b = np.asarray(proj_b, dtype=np.float32)

    B = x.shape[0]
    bpc = B // 8
    if 'nc' not in _cache:
        _cache['nc'] = build_program(bpc)
    nc = _cache['nc']

    in_maps, bpc = _marshal(x, qkv_w, q_bias, v_bias, rpb_table,
                            proj_w, proj_b, rel_index)
    res = run_bass_kernel_spmd(nc, in_maps, core_ids=list(range(8)),
                               trace=bool(os.environ.get("KERNEL_TRACE")))
    last_exec_time_ns = res.exec_time_ns
    last_results = res
    ys = [res.results[c]["y"].reshape(bpc, N_TOK, DIM) for c in range(8)]
    return np.concatenate(ys, axis=0).astype(np.float32)
